# revision 27
# baseline (speedup 1.0000x reference)
"""Bahdanau attention Trainium2 Bass kernel.

Full inputs -> full outputs. Data-parallel over batch across 8 NeuronCores,
weights replicated. Per core (32 batches):

  score^T = tanh(W1^T @ X^T + proj_h broadcast + biases)   [U=1024, T=2048]
  logits  = V^T @ score^T                                  [1, T]
  attn    = softmax_p(logits)                              per batch (P=64)
  context = sum_p attn * features                          [b, F]

Matmuls run in float32r (single-pass fp32, ~1e-3 rel err). Features are
transposed on the PE (128x128 tiles, 4 per PSUM bank, one DVE eviction per
bank); the transposes of chunk c+1 are woven between the matmul groups of
chunk c so the DVE evictions spread across the whole phase. proj_h
(hidden @ W2 + biases) is folded into the main matmul PSUM accumulation via
a block one-hot matrix. Softmax runs in row layout with step-0 broadcast
APs. Context is computed on the PE from natural-layout feature tiles with a
block-diagonal attention operand that is built on-chip (rank-1 matmul + two
partition-aligned copies), emitted after the next chunk's transposes so the
PE never waits on the softmax chain.

DMA queues: big streaming loads on the SP HWDGE ring (nc.sync), weights on
the ACT HWDGE ring (nc.scalar), small per-chunk shuffles on SWDGE
(nc.gpsimd) so nothing queues behind the big loads.
"""

import os
import numpy as np

import concourse.bass as bass
import concourse.bacc as bacc
import concourse.tile as tile
from concourse import mybir
from concourse import bass_utils


# Problem shapes (hardcoded per spec)
B, P, F, U = 256, 64, 2048, 1024
NCORES = 8
BC = B // NCORES          # 32 batches per core
CB = 4                    # batches per chunk
NCH = BC // CB            # 8 chunks
TC = CB * P               # 256 rows (t) per chunk
TT = TC // 128            # 2 t-tiles per chunk
FT = F // 128             # 16 f-tiles
UT = U // 128             # 8 u-tiles
RT = U // 128             # 8 k-tiles for W2 (rnn units = 1024)

f32 = mybir.dt.float32
f32r = mybir.dt.float32r
AX = mybir.AxisListType
AF = mybir.ActivationFunctionType

_CACHE = {}


def _build():
    nc = bacc.Bacc("TRN2", target_bir_lowering=False, debug=False,
                   num_devices=NCORES)

    feat_d = nc.dram_tensor("features", [BC, P, F], f32r, kind="ExternalInput").ap()
    hid_d = nc.dram_tensor("hidden", [BC, U], f32r, kind="ExternalInput").ap()
    w1_d = nc.dram_tensor("W1_w", [F, U], f32r, kind="ExternalInput").ap()
    nc.dram_tensor("W1_b", [1, U], f32, kind="ExternalInput")
    w2_d = nc.dram_tensor("W2_w", [U, U], f32r, kind="ExternalInput").ap()
    nc.dram_tensor("W2_b", [1, U], f32, kind="ExternalInput")
    b12_d = nc.dram_tensor("b12_c", [1, U], f32r, kind="ExternalInput").ap()
    v_d = nc.dram_tensor("V_w", [U, 1], f32r, kind="ExternalInput").ap()
    # V_b shifts every logit of a batch equally; softmax is shift-invariant,
    # so it does not affect either output. Declared so the input binds.
    nc.dram_tensor("V_b", [1, 1], f32, kind="ExternalInput")
    # host-built constants
    ident_d = nc.dram_tensor("ident_c", [128, 128], f32r, kind="ExternalInput").ap()
    e8_d = nc.dram_tensor("e8_c", [CB, TC], f32r, kind="ExternalInput").ap()
    onesb_d = nc.dram_tensor("onesb_c", [1, BC], f32r, kind="ExternalInput").ap()

    ctx_d = nc.dram_tensor("context", [BC, F], f32, kind="ExternalOutput").ap()
    attn_d = nc.dram_tensor("attn", [BC, P], f32r, kind="ExternalOutput").ap()

    feat_flat = feat_d.rearrange("b p f -> (b p) f")

    with tile.TileContext(nc) as tc:
        with tc.tile_pool(name="persist", bufs=1) as pp, \
             tc.tile_pool(name="pst", bufs=2, space="PSUM") as pst, \
             tc.tile_pool(name="psmm", bufs=2, space="PSUM") as psmm, \
             tc.tile_pool(name="pslg", bufs=2, space="PSUM") as pslg, \
             tc.tile_pool(name="psct", bufs=2, space="PSUM") as psct:

            # SP ring first: identity + hidden (tiny, unblock first PE work)
            ident = pp.tile([128, 128], f32r)
            nc.sync.dma_start(out=ident, in_=ident_d)

            # ACT ring: small constants, then W2 (needed early), then W1
            vt = pp.tile([128, UT], f32r)
            nc.scalar.dma_start(out=vt, in_=v_d.rearrange("(ut up) o -> up (ut o)", up=128))
            e8 = pp.tile([CB, TC], f32r)
            nc.scalar.dma_start(out=e8, in_=e8_d)
            onesb = pp.tile([1, BC], f32r)
            nc.scalar.dma_start(out=onesb, in_=onesb_d)
            brow = pp.tile([1, U], f32r)
            nc.scalar.dma_start(out=brow, in_=b12_d)
            zrow = pp.tile([128, 2 * CB], f32)
            nc.vector.memset(zrow, 0.0)

            hscr = pp.tile([BC, U], f32r)
            wp = tc.alloc_tile_pool(name="w2pool", bufs=1)
            w2 = wp.tile([128, RT, U], f32r)
            nc.scalar.dma_start(out=w2, in_=w2_d.rearrange("(rt rp) u -> rp rt u", rp=128))
            w1 = pp.tile([128, FT, U], f32r)
            for q in range(4):
                nc.scalar.dma_start(
                    out=w1[:, q * 4:(q + 1) * 4, :],
                    in_=w1_d.rearrange("(ft fp) u -> fp ft u", fp=128)[:, q * 4:(q + 1) * 4, :],
                )
            hload = wp.tile([BC, U], f32r)
            nc.sync.dma_start(out=hload, in_=hid_d)

            # hidden transpose + proj_h: hscr[b, u] = hidden @ W2 + brow
            hT = wp.tile([128, RT, BC], f32r)
            for rt in range(RT):
                ptr = pst.tile([128, 512], f32r, tag="tr")
                nc.tensor.transpose(ptr[:, :BC], hload[:, rt * 128:(rt + 1) * 128],
                                    ident[:BC, :BC])
                nc.vector.tensor_copy(hT[:, rt, :], ptr[:, :BC])
            for uc in range(2):
                ph = psmm.tile([BC, 512], f32, tag="mm")
                for rt in range(RT):
                    nc.tensor.matmul(ph, hT[:, rt, :], w2[:, rt, uc * 512:(uc + 1) * 512],
                                     start=(rt == 0), stop=False)
                nc.tensor.matmul(ph, onesb, brow[:, uc * 512:(uc + 1) * 512],
                                 start=False, stop=True)
                nc.vector.tensor_copy(hscr[:, uc * 512:(uc + 1) * 512], ph)
            wp.release()

            # ---- chunked, software-pipelined main loop ----
            with tc.tile_pool(name="xnat", bufs=4) as xnp, \
                 tc.tile_pool(name="xtp", bufs=2) as xtp, \
                 tc.tile_pool(name="scp", bufs=2) as scp, \
                 tc.tile_pool(name="cxp", bufs=1) as cxp, \
                 tc.tile_pool(name="smp", bufs=2) as smp:

                state = {}

                def prep(c):
                    # allocate xt + load this chunk's feature tiles
                    xt_c = xtp.tile([128, FT, TC], f32r, tag="xt")
                    xns = []
                    for tt in range(TT):
                        xn = xnp.tile([128, F], f32r, tag="xn")
                        r0 = c * TC + tt * 128
                        nc.sync.dma_start(out=xn, in_=feat_flat[r0:r0 + 128, :])
                        xns.append(xn)
                    state[c] = {"xt": xt_c, "xns": xns}

                def trquad(c, q):
                    # transpose 4 f-tiles of one t-tile into one PSUM bank,
                    # evict with a single DVE copy
                    st = state[c]
                    tt, fq = q // (FT // 4), q % (FT // 4)
                    xn = st["xns"][tt]
                    ptr = pst.tile([128, 512], f32r, tag="tr")
                    for k in range(4):
                        ft = fq * 4 + k
                        nc.tensor.transpose(ptr[:, k * 128:(k + 1) * 128],
                                            xn[:, ft * 128:(ft + 1) * 128], ident)
                    nc.vector.tensor_copy(
                        st["xt"][:, fq * 4:(fq + 1) * 4, tt * 128:(tt + 1) * 128],
                        ptr.rearrange("q (a b) -> q a b", a=4))

                def mainblock(c, trc):
                    st = state[c]
                    xt_c = st["xt"]
                    ph_c = smp.tile([CB, U], f32r, tag="ph")
                    nc.gpsimd.dma_start(out=ph_c, in_=hscr[c * CB:(c + 1) * CB, :])

                    sc_c = scp.tile([128, UT, TC], f32r, tag="sc")
                    for ut in range(UT):
                        if trc is not None:
                            trquad(trc, ut)
                        pmm = psmm.tile([128, TC], f32, tag="mm")
                        for ft in range(FT):
                            nc.tensor.matmul(pmm, w1[:, ft, ut * 128:(ut + 1) * 128],
                                             xt_c[:, ft, :], start=(ft == 0), stop=False)
                        nc.tensor.matmul(pmm, ph_c[:, ut * 128:(ut + 1) * 128], e8,
                                         start=False, stop=True)
                        nc.scalar.activation(sc_c[:, ut, :], pmm, AF.Tanh)

                    plg = pslg.tile([1, TC], f32, tag="lg")
                    for ut in range(UT):
                        nc.tensor.matmul(plg, vt[:, ut:ut + 1], sc_c[:, ut, :],
                                         start=(ut == 0), stop=(ut == UT - 1))

                    # softmax over each 64-wide segment, in row layout,
                    # reading logits straight from PSUM
                    def seg_bc(t):
                        return bass.AP(tensor=t.tensor, offset=t.offset,
                                       ap=[t.ap[0], [t.ap[1][0], CB], [0, P]])

                    lgv = plg.rearrange("o (s i) -> o s i", s=CB)
                    neg4 = smp.tile([1, CB], f32, tag="nm")
                    nc.vector.reduce_max(neg4, lgv, axis=AX.X, negate=True)
                    shf = smp.tile([1, TC], f32, tag="shf")
                    shv = shf.rearrange("o (s i) -> o s i", s=CB)
                    nc.vector.tensor_tensor(out=shv, in0=lgv, in1=seg_bc(neg4),
                                            op=mybir.AluOpType.add)
                    exr = smp.tile([1, TC], f32, tag="exr")
                    nc.scalar.activation(exr, shf, AF.Exp)
                    exv = exr.rearrange("o (s i) -> o s i", s=CB)
                    sum4 = smp.tile([1, CB], f32, tag="se")
                    nc.vector.reduce_sum(sum4, exv, axis=AX.X)
                    r4 = smp.tile([1, CB], f32, tag="rs")
                    nc.vector.reciprocal(r4, sum4)
                    attn_row = smp.tile([1, TC], f32r, tag="arow")
                    nc.vector.tensor_tensor(out=attn_row.rearrange("o (s i) -> o s i", s=CB),
                                            in0=exv, in1=seg_bc(r4),
                                            op=mybir.AluOpType.mult)
                    nc.gpsimd.dma_start(out=attn_d[c * CB:(c + 1) * CB, :], in_=attn_row)
                    st["attn_row"] = attn_row

                def ctxblock(c):
                    st = state.pop(c)
                    attn_row = st["attn_row"]
                    xns = st["xns"]
                    # block-diagonal attention operand built on-chip: rank-1
                    # matmul turns each 128-wide row slice into a column, then
                    # two partition-aligned DVE copies place the 64-row blocks.
                    bd = smp.tile([128, 2 * CB], f32r, tag="bd")
                    nc.vector.tensor_copy(bd, zrow)
                    for tt in range(TT):
                        pac = pst.tile([128, 512], f32r, tag="tr")
                        nc.tensor.matmul(pac[:, :4].bitcast(f32),
                                         attn_row[:, tt * 128:(tt + 1) * 128].bitcast(f32),
                                         onesb[:, :4].bitcast(f32), start=True, stop=True)
                        for m in range(2):
                            nc.vector.tensor_copy(
                                bd[m * P:(m + 1) * P,
                                   tt * CB + 2 * tt + m:tt * CB + 2 * tt + m + 1],
                                pac[m * P:(m + 1) * P, :1].bitcast(f32))
                    csb = cxp.tile([CB, F], f32, tag="csb")
                    for fc in range(4):
                        pct = psct.tile([CB, 512], f32, tag="ct")
                        for tt in range(TT):
                            nc.tensor.matmul(pct, bd[:, tt * CB:(tt + 1) * CB],
                                             xns[tt][:, fc * 512:(fc + 1) * 512],
                                             start=(tt == 0), stop=(tt == TT - 1))
                        nc.vector.tensor_copy(csb[:, fc * 512:(fc + 1) * 512], pct)
                    nc.gpsimd.dma_start(out=ctx_d[c * CB:(c + 1) * CB, :], in_=csb)

                # chunks 0 and 1 are transposed during the startup window
                # while the PE would otherwise idle waiting for W1
                prep(0)
                for q in range(UT):
                    trquad(0, q)
                prep(1)
                for q in range(UT):
                    trquad(1, q)
                mainblock(0, None)
                ctxblock(0)
                for c in range(1, NCH - 1):
                    prep(c + 1)
                    mainblock(c, c + 1)
                    ctxblock(c)
                mainblock(NCH - 1, None)
                ctxblock(NCH - 1)

    nc.compile()
    return nc


def kernel(features, hidden, W1_w, W1_b, W2_w, W2_b, V_w, V_b):
    if "nc" not in _CACHE:
        _CACHE["nc"] = _build()
    nc = _CACHE["nc"]

    f32n = np.float32
    features = np.ascontiguousarray(features, dtype=f32n)
    hidden = np.ascontiguousarray(hidden, dtype=f32n)
    shared = {
        "W1_w": np.ascontiguousarray(W1_w, dtype=f32n),
        "W1_b": np.ascontiguousarray(W1_b, dtype=f32n).reshape(1, U),
        "W2_w": np.ascontiguousarray(W2_w, dtype=f32n),
        "W2_b": np.ascontiguousarray(W2_b, dtype=f32n).reshape(1, U),
        "V_w": np.ascontiguousarray(V_w, dtype=f32n).reshape(U, 1),
        "V_b": np.ascontiguousarray(V_b, dtype=f32n).reshape(1, 1),
        "b12_c": (np.asarray(W1_b, dtype=f32n) + np.asarray(W2_b, dtype=f32n)).reshape(1, U),
        "ident_c": np.eye(128, dtype=f32n),
        "e8_c": np.repeat(np.eye(CB, dtype=f32n), P, axis=1),
        "onesb_c": np.ones((1, BC), dtype=f32n),
    }
    in_maps = []
    for i in range(NCORES):
        s = slice(i * BC, (i + 1) * BC)
        m = {"features": features[s], "hidden": hidden[s]}
        m.update(shared)
        in_maps.append(m)

    trace = bool(int(os.environ.get("ATT_TRACE", "0")))
    res = None
    last_err = None
    for _attempt in range(4):
        try:
            res = bass_utils.run_bass_kernel_spmd(
                nc, in_maps, core_ids=list(range(NCORES)), trace=trace)
            break
        except Exception as e:  # transient NRT_EXEC_UNIT errors on fresh NEFFs
            last_err = e
            if isinstance(e, (ImportError, ModuleNotFoundError)):
                # externally-set BASS_TRACE hits the missing axon NTFF hook;
                # force the no-trace path and retry
                os.environ["BASS_NEVER_TRACE"] = "1"
                trace = False
    if res is None:
        raise last_err
    _CACHE["last_exec_time_ns"] = res.exec_time_ns

    context = np.concatenate([res.results[i]["context"] for i in range(NCORES)], axis=0)
    attn = np.concatenate([res.results[i]["attn"] for i in range(NCORES)], axis=0)
    return context, attn.reshape(B, P, 1)
